# revision 30
# baseline (speedup 1.0000x reference)
"""Trainium2 Bass kernel for nn_Downsample (depthwise 4x4 FIR, stride 2).

Strategy: data-parallel over batch (8 cores, one batch element each).
The whole separable FIR runs on the tensor engine as a sum of 8 band-matrix
matmuls per PSUM region:

  out[h',c,w'] = sum_{i,ph} (g_w[i] * A_ph)^T  X[ph, :, c, par(i), w'+off(i)]

The host pre-transposes x to [h-phase, h/2, c, w-parity, w/2] so that
every DMA descriptor is fully contiguous per partition (~32 KB runs) and
each W-tap of the output is an aligned (or 1-shifted) slice of the moving
operand.  The H-FIR lives in the polyphase band matrices A_e/A_o
[128x128]; the W-FIR taps become 8 pre-scaled stationary matrices
(g_w[i] * A_ph) accumulated into the same PSUM region, with the two
shifted taps writing partial column ranges (which also handles the W
edges exactly).  Stage 2 is then just a PSUM -> SBUF fp16 copy,
alternated between the scalar and vector engines.

fp16 input/output halves DMA bytes; accumulation is fp32 in PSUM.
"""

import numpy as np

B, C, H, W = 8, 256, 256, 256
HO, WO = H // 2, W // 2
N_CORES = 8
TAPS = 4
PAD0 = 1          # (kh - factor + 1) // 2 for kh=4, factor=2
CB = 32           # channels per DMA block
PT = 8            # channels per PSUM tile (2 banks, 4 ch/bank)

_CACHE = {}


def _band_matrix(g, n_in, n_out):
    """A[h, h'] = g[i] at h = 2*h' - PAD0 + i, zero-padded at the edges."""
    a = np.zeros((n_in, n_out), dtype=np.float32)
    for hp in range(n_out):
        for i in range(TAPS):
            h = 2 * hp - PAD0 + i
            if 0 <= h < n_in:
                a[h, hp] = g[i]
    return a


def _build_program():
    from concourse import bacc, tile
    import concourse.mybir as mybir

    R = mybir.dt.float16
    F32 = mybir.dt.float32

    MULT = mybir.AluOpType.mult
    ADD = mybir.AluOpType.add
    COPY = mybir.ActivationFunctionType.Copy

    nc = bacc.Bacc("TRN2", target_bir_lowering=False, debug=False,
                   num_devices=N_CORES)
    # host-prepped layouts: x (h-phase, h/2, c, w-parity, w/2)
    x_d = nc.dram_tensor("x", [2, HO, C, 2, WO], R, kind="ExternalInput").ap()
    # amat is host-prepped partition-major: fully contiguous DMA
    am_d = nc.dram_tensor("amat", [HO, 8, HO], R, kind="ExternalInput").ap()
    y_d = nc.dram_tensor("y", [HO, C, WO], R, kind="ExternalOutput").ap()

    n_cblk = C // CB
    n_pt = CB // PT

    # W-tap schedule: (stationary idx s = 2*i + ph, h-phase, w-parity,
    #                  out w' range, in w' range)
    # tap i=1 first (start=True, full range), tap i=2 last (stop=True, full).
    FULL = (0, WO)
    MM_PLAN = [
        (2, 0, 0, FULL, FULL),             # i=1: T_even aligned
        (3, 1, 0, FULL, FULL),
        (0, 0, 1, (1, WO), (0, WO - 1)),   # i=0: T_odd shifted left
        (1, 1, 1, (1, WO), (0, WO - 1)),
        (6, 0, 0, (0, WO - 1), (1, WO)),   # i=3: T_even shifted right
        (7, 1, 0, (0, WO - 1), (1, WO)),
        (4, 0, 1, FULL, FULL),             # i=2: T_odd aligned
        (5, 1, 1, FULL, FULL),
    ]

    with tile.TileContext(nc) as tc:
        with tc.tile_pool(name="const", bufs=1) as constp, \
             tc.tile_pool(name="xin", bufs=4) as xinp, \
             tc.tile_pool(name="outp", bufs=3) as outp, \
             tc.tile_pool(name="ps", bufs=4, space="PSUM") as psp:

            am_t = constp.tile([128, 8, HO], R)
            nc.scalar.dma_start(out=am_t[:], in_=am_d)

            tidx = 0
            for cb in range(n_cblk):
                c0 = cb * CB
                xt = xinp.tile([128, 2, CB, 2, WO], R, tag="x")
                src = x_d[:, :, c0:c0 + CB, :, :]
                if cb == 0:
                    # ramped first load so the first matmuls start early
                    for qs, qe in ((0, 4), (4, 8), (8, 16), (16, 32)):
                        nc.sync.dma_start(
                            out=xt[:, :, qs:qe, :, :],
                            in_=src[:, :, qs:qe, :, :].rearrange(
                                "k p c v w -> p k c v w"))
                    tiles = ((0, 4), (4, 8), (8, 16), (16, 24), (24, 32))
                else:
                    nc.sync.dma_start(
                        out=xt[:], in_=src.rearrange("k p c v w -> p k c v w"))
                    if cb == n_cblk - 1:
                        # smaller final tiles drain the tail sooner
                        tiles = ((0, 8), (8, 16), (16, 24), (24, 28), (28, 32))
                    else:
                        tiles = ((0, 8), (8, 16), (16, 24), (24, 32))
                ot = outp.tile([128, CB, WO], R, tag="out")

                for s0, s1 in tiles:
                    span = s1 - s0
                    ps = psp.tile([128, PT, WO], F32)
                    for mi, (s, ph, vv, (o0, o1), (i0, i1)) in enumerate(MM_PLAN):
                        for bk in range(span // 4):
                            cc = s0 + 4 * bk
                            nc.tensor.matmul(
                                ps[:, 4 * bk:4 * bk + 4, o0:o1],
                                am_t[:, s, :],
                                xt[:, ph, cc:cc + 4, vv, i0:i1],
                                start=(mi == 0), stop=(mi == len(MM_PLAN) - 1),
                                skip_group_check=True)

                    # stage 2 collapsed: plain PSUM -> SBUF fp16 copy
                    if tidx % 2 == 0:
                        nc.scalar.copy(ot[:, s0:s1, :], ps[:, 0:span, :])
                    else:
                        nc.vector.tensor_copy(ot[:, s0:s1, :], ps[:, 0:span, :])
                    tidx += 1

                    if cb == n_cblk - 1:
                        # drain the last block as soon as each slice is ready
                        nc.scalar.dma_start(
                            out=y_d[:, c0 + s0:c0 + s1, :],
                            in_=ot[:, s0:s1, :])

                if cb < n_cblk - 1:
                    nc.scalar.dma_start(out=y_d[:, c0:c0 + CB, :], in_=ot[:])

    nc.compile()
    return nc


def _get_program():
    if "nc" not in _CACHE:
        _CACHE["nc"] = _build_program()
    return _CACHE["nc"]


def kernel(x, kernel):
    from concourse.bass_utils import run_bass_kernel_spmd

    x = np.asarray(x, dtype=np.float32)
    k = np.asarray(kernel, dtype=np.float32)

    # reference correlates with the flipped kernel; separable factors from
    # row/col sums (exact for normalized separable kernels)
    w = k[::-1, ::-1].astype(np.float64)
    g_h = w.sum(axis=1)
    g_w = w.sum(axis=0)
    s = w.sum()
    if not np.isclose(s, 1.0):
        g_h = g_h / np.sqrt(s)
        g_w = g_w / np.sqrt(s)
    g_h = g_h.astype(np.float32)
    g_w = g_w.astype(np.float32)

    a_h = _band_matrix(g_h, H, HO)                       # [H, HO]
    # polyphase split: A_ph[p, m] = a_h[2p + ph, m]
    a_ph = a_h.reshape(HO, 2, HO)                        # [p, ph, m]
    amat = np.empty((8, HO, HO), dtype=np.float16)
    for i in range(TAPS):
        for ph in range(2):
            amat[2 * i + ph] = (g_w[i] * a_ph[:, ph, :]).astype(np.float16)
    amat = np.ascontiguousarray(amat.transpose(1, 0, 2))  # [p, s, m]

    nc = _get_program()
    in_maps = []
    for b in range(B):
        xb = x[b].astype(np.float16)                     # [C, H, W]
        # -> [h-phase, h/2, c, w-parity, w/2]
        xp = np.ascontiguousarray(
            xb.reshape(C, HO, 2, WO, 2).transpose(2, 1, 0, 4, 3))
        in_maps.append({"x": xp, "amat": amat})

    res = run_bass_kernel_spmd(nc, in_maps, core_ids=list(range(N_CORES)))
    _CACHE["last_result"] = res
    out = np.stack(
        [res.results[b]["y"].transpose(1, 0, 2) for b in range(B)], axis=0)
    return out.astype(np.float32)
